# revision 72
# baseline (speedup 1.0000x reference)
"""Trainium2 Bass kernel for multi-head attention with RoPE.

Problem: b=8, n=1024, d_model=768, heads=12, dim_head=64.
Strategy: data parallel over batch — each of the 8 NeuronCores handles one
batch element end-to-end (QKV proj + RoPE + attention + out proj). No
collectives needed.

Per-core math (all in transposed [feature, token] layout so every matmul
contraction sits on the partition axis; all matmul operands padded to the
full 128 partitions for full SBUF-stream bandwidth):
  xT   [768,1024]  = x^T             (bf16, via DMA transpose)
  qT   [768,1024]  = Wq^T x^T        then RoPE in bf16 on DVE
  kz   2x[128,1024] per head pair: rotated k rows zero-padded to K=128
  V    [1024,12*128] = x Wv, 128 cols/head: 64 v | ones col | zeros
  per head pair (software-pipelined with the next S/exp step):
    sT[j,i] = sum_d kz[d,j] qT[d,i]  (K=128 contraction, zeros inert)
    pT  = exp(sT / 8)                (no max-subtraction; |S/8| <~ 6)
    oT[128,1024] += Vaug^T-style accum over j tiles; row 64 = softmax
                    denominators (ones column), rows 65+ zeros
    aT = oT[0:64] * bcast(1/oT[64])  (recip_approx + DRAM-broadcast DMA)
  out [1024,768] = aT^T Wout + b
"""

import os
import numpy as np
import ml_dtypes

N = 1024
D = 768
H = 12
DH = 64
E3 = 2304
KT = 6          # number of 128-row tiles of the model dim (768/128)
NT = 8          # number of 128-token tiles (1024/128)
P = 128
N_CORES = 8
VW = 65         # per-head V width incl. ones column

_CACHE = {}


def _build():
    import concourse.bass as bass
    import concourse.mybir as mybir
    import concourse.tile as tile
    from concourse import bacc

    F32 = mybir.dt.float32
    BF16 = mybir.dt.bfloat16
    Exp = mybir.ActivationFunctionType.Exp

    nc = bacc.Bacc("TRN2", target_bir_lowering=False, debug=False,
                   num_devices=N_CORES)

    x = nc.dram_tensor("x", [N, D], BF16, kind="ExternalInput")
    wqkv = nc.dram_tensor("wqkv", [D, E3], BF16, kind="ExternalInput")
    wout = nc.dram_tensor("wout", [D, D], BF16, kind="ExternalInput")
    cos2 = nc.dram_tensor("cos2", [P, N], BF16, kind="ExternalInput")
    sins2 = nc.dram_tensor("sins2", [P, N], BF16, kind="ExternalInput")
    biasb = nc.dram_tensor("biasb", [1, D], BF16, kind="ExternalInput")
    out = nc.dram_tensor("out", [N, D], F32, kind="ExternalOutput")

    with tile.TileContext(nc, pool_alloc_mode="queue") as tc:
        import contextlib
        with contextlib.ExitStack() as ctx:
            persist = ctx.enter_context(tc.tile_pool(name="persist", bufs=1))
            scr = ctx.enter_context(tc.tile_pool(name="scr", bufs=6))
            ptp = ctx.enter_context(tc.tile_pool(name="ptp", bufs=4))
            smallp = ctx.enter_context(tc.tile_pool(name="smallp", bufs=2))
            otp = ctx.enter_context(tc.tile_pool(name="otp", bufs=4))
            outp = ctx.enter_context(tc.tile_pool(name="outp", bufs=2))
            dramp = ctx.enter_context(
                tc.tile_pool(name="dram", bufs=2, space="DRAM"))

            # ---- startup loads, ordered so the first V matmul's
            # dependencies (transpose 0 + v-columns) land first
            xT = [persist.tile([P, N], BF16, tag=f"xT{t_i}",
                               name=f"xT_sb{t_i}") for t_i in range(KT)]
            nc.sync.dma_start_transpose(xT[0][:], x[:, 0:P])
            wv_sb = persist.tile([P, KT * D], BF16, tag="wv", name="wv_sb")
            nc.sync.dma_start(
                wv_sb[:].rearrange("p (k e) -> p k e", k=KT),
                wqkv[:, 1536:E3].rearrange("(k p) e -> p k e", p=P))
            for t_i in range(1, KT):
                nc.sync.dma_start_transpose(xT[t_i][:],
                                            x[:, t_i * P:(t_i + 1) * P])
            wqk_sb = persist.tile([P, KT * 1536], BF16, tag="wqk",
                                  name="wqk_sb")
            for k in range(KT):
                nc.sync.dma_start(wqk_sb[:, k * 1536:(k + 1) * 1536],
                                  wqkv[k * P:(k + 1) * P, 0:1536])
            cos_sb = persist.tile([P, N], BF16, tag="cos", name="cos_sb")
            nc.sync.dma_start(cos_sb[:], cos2[:, :])
            sin_sb = persist.tile([P, N], BF16, tag="sin", name="sin_sb")
            nc.sync.dma_start(sin_sb[:], sins2[:, :])
            wo_sb = persist.tile([P, KT * D], BF16, tag="wo", name="wo_sb")
            nc.sync.dma_start(
                wo_sb[:].rearrange("p (k e) -> p k e", k=KT),
                wout[:, :].rearrange("(k p) e -> p k e", p=P))
            bias_sb = persist.tile([1, D], BF16, tag="bias", name="bias_sb")
            nc.sync.dma_start(bias_sb[:], biasb[:, :])
            ones1_sb = persist.tile([1, P], BF16, tag="ones1",
                                    name="ones1_sb")
            nc.gpsimd.memset(ones1_sb[:], 1.0)

            # =======================================================
            # Phase B/C: projections (shared PSUM pool, closed after)
            # =======================================================
            qkT = [persist.tile([P, N], BF16, tag=f"qkT{m}", name=f"qkT_sb{m}")
                   for m in range(6)]
            # zero-padded K tiles: kz[hp][u] holds head 2hp+u's rotated k
            # rows in their natural 64-row half, zeros in the other half,
            # so S^T matmuls contract a full K=128 (full SBUF stream BW).
            kz = [[persist.tile([P, N], BF16, tag=f"kz{hp}_{u}",
                                name=f"kz_sb{hp}_{u}") for u in range(2)]
                  for hp in range(6)]
            # V tiles padded to 128 cols/head: 64 v-dims | ones | zeros,
            # so PV matmuls write a full M=128 (ones col -> sums row 64).
            vt = [persist.tile([P, H * P], BF16, tag=f"vt{n}", name=f"vt_sb{n}")
                  for n in range(NT)]
            aT = [persist.tile([P, N], BF16, tag=f"aT{e}", name=f"aT_sb{e}")
                  for e in range(KT)]
            for hp in range(6):
                nc.gpsimd.memset(kz[hp][0][DH:P, :], 0.0)
                nc.gpsimd.memset(kz[hp][1][0:DH, :], 0.0)

            # Two dedicated PSUM pools (2 slots x 2 banks each = all 8
            # banks): psS cycles projection / S^T / final tiles, psO holds
            # the two PV accumulators. Separate pools make the slot
            # handoff at head-pair boundaries deterministic: fresh S tiles
            # always reuse S slots (freed by exp), PV accumulators reuse
            # the old accumulators (drained by the oT copy during the
            # first S/exp of the new pair).
            with (tc.tile_pool(name="psS", bufs=2, space="PSUM") as psum,
                  tc.tile_pool(name="psO", bufs=2, space="PSUM") as psumO):
                # ---- V projection into per-head 65-wide layout ----
                for ni in range(NT):
                    vpool, vtag = ((psum, "ps") if ni % 2 == 0
                                   else (psumO, "ops"))
                    ps = vpool.tile([P, N], F32, tag=vtag, name="ps_v")
                    for (c0, cw) in ((0, 512), (512, 256)):
                        for k in range(KT):
                            nc.tensor.matmul(
                                ps[:, c0:c0 + cw],
                                lhsT=xT[k][:, ni * P:(ni + 1) * P],
                                rhs=wv_sb[:, k * D + c0:k * D + c0 + cw],
                                start=(k == 0), stop=(k == KT - 1))
                    # scatter copy into head-strided slots (+ones col)
                    dst8 = vt[ni][:, 0:8 * P].rearrange(
                        "p (h j) -> p h j", j=P)[:, :, 0:DH]
                    src8 = ps[:, 0:512].rearrange("p (h j) -> p h j", j=DH)
                    nc.scalar.copy(dst8, src8)
                    dst4 = vt[ni][:, 8 * P:12 * P].rearrange(
                        "p (h j) -> p h j", j=P)[:, :, 0:DH]
                    src4 = ps[:, 512:768].rearrange("p (h j) -> p h j", j=DH)
                    nc.scalar.copy(dst4, src4)
                    vre = vt[ni].rearrange("p (h j) -> p h j", j=P)
                    nc.gpsimd.memset(vre[:, :, DH:DH + 1], 1.0)
                    nc.gpsimd.memset(vre[:, :, DH + 1:P], 0.0)

                # ---- q/k projection + RoPE (order: head-pair hp first
                # needs tiles hp and 6+hp, so emit in that pairing) ----
                for mi, m in enumerate(
                        [t for hp in range(6) for t in (hp, 6 + hp)]):
                    qpool, qtag = ((psum, "ps") if mi % 2 == 0
                                   else (psumO, "ops"))
                    ps = qpool.tile([P, N], F32, tag=qtag, name="ps_qk")
                    for ih in range(2):
                        for k in range(KT):
                            nc.tensor.matmul(
                                ps[:, ih * 512:(ih + 1) * 512],
                                lhsT=wqk_sb[:, k * 1536 + m * P:
                                            k * 1536 + (m + 1) * P],
                                rhs=xT[k][:, ih * 512:(ih + 1) * 512],
                                start=(k == 0), stop=(k == KT - 1))
                    # RoPE in bf16 (DVE 2x_1p mode)
                    qf = scr.tile([P, N], BF16, tag="qf", name="qf_t")
                    nc.scalar.copy(qf[:], ps[:])
                    qa = scr.tile([P, N], BF16, tag="qa", name="qa_t")
                    nc.vector.tensor_mul(qa[:], qf[:], cos_sb[:])
                    qb = scr.tile([P, N], BF16, tag="qb", name="qb_t")
                    # sin table is pre-swapped on host so in0/in1 share a
                    # base partition (walrus NCC_IBIR297); only the output
                    # is quadrant-shifted. Two of the four quadrant muls
                    # go to the otherwise-idle GpSimd engine — the rope is
                    # DVE-bound otherwise.
                    for blk in range(4):
                        ob = blk * 32
                        ib = (blk ^ 1) * 32  # 0<->32, 64<->96
                        nc.vector.tensor_mul(
                            qb[ob:ob + 32, :], qf[ib:ib + 32, :],
                            sin_sb[ib:ib + 32, :])
                    if m < 6:
                        nc.vector.tensor_add(qkT[m][:], qa[:], qb[:])
                    else:
                        hp = m - 6
                        nc.vector.tensor_add(
                            kz[hp][0][0:DH, :], qa[0:DH, :], qb[0:DH, :])
                        nc.vector.tensor_add(
                            kz[hp][1][DH:P, :], qa[DH:P, :], qb[DH:P, :])

                # ---- attention, two heads (one row-group pair) at a
                # time; software-pipelined one step: emit S/exp of step
                # g+1 before PV of step g so the PE never stalls on the
                # last exp of a head pair (incl. across pair boundaries).
                ss_sb = persist.tile([33, N], F32, tag="ss", name="ss_t")
                nc.gpsimd.memset(ss_sb[0:32, :], 1.0)
                o_ps_all = [[None, None] for _ in range(6)]

                def emit_s_exp(hp, j):
                    qt = qkT[hp]
                    s_ps = [psum.tile([P, N], F32, tag="ps",
                                      name=f"s_ps{u}") for u in range(2)]
                    for ih in range(2):
                        for u in range(2):  # u: head parity
                            nc.tensor.matmul(
                                s_ps[u][:, ih * 512:(ih + 1) * 512],
                                lhsT=kz[hp][u][:, j * P:(j + 1) * P],
                                rhs=qt[:, ih * 512:(ih + 1) * 512],
                                start=True, stop=True)
                    pT = [None, None]
                    for u in range(2):
                        pT[u] = ptp.tile([P, N], BF16, tag="pT",
                                         name=f"pT_t{u}")
                        nc.scalar.activation(pT[u][:], s_ps[u][:], Exp,
                                             scale=0.125)
                    return pT

                def emit_pv(hp, j, pT):
                    o_ps = o_ps_all[hp]
                    if j == 0:
                        o_ps[0] = psumO.tile([P, N], F32, tag="ops",
                                             name="o_ps0")
                        o_ps[1] = psumO.tile([P, N], F32, tag="ops",
                                             name="o_ps1")
                    for u in range(2):
                        h = 2 * hp + u
                        for ih in range(2):
                            nc.tensor.matmul(
                                o_ps[u][:, ih * 512:(ih + 1) * 512],
                                lhsT=vt[j][:, h * P:(h + 1) * P],
                                rhs=pT[u][:, ih * 512:(ih + 1) * 512],
                                start=(j == 0), stop=(j == NT - 1))
                    if j == NT - 1:
                        emit_normalize(hp, o_ps)

                def emit_normalize(hp, o_ps):
                    # copy attn + sums rows out fast (frees o_ps), then
                    # 1/sums + DRAM broadcast + one scale multiply run in
                    # parallel with the next pair's matmuls.
                    oTc = otp.tile([P, N], BF16, tag="oT", name="oT_t")
                    if hp == 5:
                        # last pair: the reciprocal chain gates phase E —
                        # start it first, oTc copies overlap the DMA RT
                        nc.vector.tensor_copy(ss_sb[0:1, :],
                                              o_ps[0][DH:DH + 1, :])
                        nc.vector.tensor_copy(ss_sb[32:33, :],
                                              o_ps[1][DH:DH + 1, :])
                        r_sb = smallp.tile([33, N], F32, tag="r",
                                           name="r_t")
                        nc.vector.reciprocal_approx_fast(r_sb[:], ss_sb[:])
                        nc.scalar.copy(oTc[0:DH, :], o_ps[0][0:DH, :])
                        nc.vector.tensor_copy(oTc[DH:P, :],
                                              o_ps[1][0:DH, :])
                    else:
                        # mid-D: both copies on DVE so ACT's stream stays
                        # pure exp (an ACT copy here waits on the last PV
                        # and delays the next pair's exps by ~1us)
                        nc.vector.tensor_copy(oTc[0:DH, :],
                                              o_ps[0][0:DH, :])
                        nc.vector.tensor_copy(oTc[DH:P, :],
                                              o_ps[1][0:DH, :])
                        nc.vector.tensor_copy(ss_sb[0:1, :],
                                              o_ps[0][DH:DH + 1, :])
                        nc.vector.tensor_copy(ss_sb[32:33, :],
                                              o_ps[1][DH:DH + 1, :])
                        r_sb = smallp.tile([33, N], F32, tag="r",
                                           name="r_t")
                        nc.vector.reciprocal_approx_fast(r_sb[:], ss_sb[:])
                    r_dr = dramp.tile([2, N], F32, tag="rdr", name="rdr_t")
                    nc.sync.dma_start(r_dr[0:1, :], r_sb[0:1, :])
                    nc.sync.dma_start(r_dr[1:2, :], r_sb[32:33, :])
                    rb_sb = smallp.tile([P, N], F32, tag="rb", name="rb_t")
                    nc.sync.dma_start(rb_sb[0:DH, :],
                                      r_dr[0:1, :].broadcast_to([DH, N]))
                    nc.sync.dma_start(rb_sb[DH:P, :],
                                      r_dr[1:2, :].broadcast_to([DH, N]))
                    nc.vector.tensor_mul(aT[hp][:], oTc[:], rb_sb[:])

                steps = [(hp, j) for hp in range(6) for j in range(NT)]
                prev = None
                for st in steps:
                    pT = emit_s_exp(*st)
                    if prev is not None:
                        emit_pv(*prev)
                    prev = (st[0], st[1], pT)
                emit_pv(*prev)

                # ---- output projection + bias (alternate pools so four
                # PSUM regions are available and e<5 accumulation hoists
                # into the last head pair's attention) ----
                for it in range(NT):
                    f_pool = psumO if it % 2 == 0 else psum
                    f_ps = f_pool.tile([P, N], F32,
                                       tag="ops" if it % 2 == 0 else "ps",
                                       name="f_ps")
                    for (c0, cw) in ((0, 512), (512, 256)):
                        for e in range(KT):
                            nc.tensor.matmul(
                                f_ps[:, c0:c0 + cw],
                                lhsT=aT[e][:, it * P:(it + 1) * P],
                                rhs=wo_sb[:, e * D + c0:e * D + c0 + cw],
                                start=(e == 0), stop=False)
                        # bias folded in as a K=1 ones-row matmul so the
                        # PSUM drain is a plain copy (ACT/DVE alternate)
                        nc.tensor.matmul(
                            f_ps[:, c0:c0 + cw],
                            lhsT=ones1_sb[:, 0:P],
                            rhs=bias_sb[:, c0:c0 + cw],
                            start=False, stop=True)
                    o_sb = outp.tile([P, D], F32, tag="osb", name="osb_t")
                    if it % 2 == 0:
                        nc.scalar.copy(o_sb[:], f_ps[:, 0:D])
                    else:
                        nc.vector.tensor_copy(o_sb[:], f_ps[:, 0:D])
                    nc.sync.dma_start(out[it * P:(it + 1) * P, :], o_sb[:])

    nc.compile()
    return nc


def _host_tables():
    inv_freq = 1.0 / (10000.0 ** (np.arange(0, DH, 2, dtype=np.float32) / DH))
    t = np.arange(N, dtype=np.float32)
    freqs = np.einsum("i,j->ij", t, inv_freq)          # [N, 32]
    emb = np.concatenate([freqs, freqs], axis=-1)      # [N, 64]
    cosT = np.cos(emb).T.astype(np.float32)            # [64, N]
    sinT = np.sin(emb).T.astype(np.float32)            # [64, N]
    # b-term: out rows 0:32 use -sin (pair d+32), rows 32:64 use +sin
    sins = np.concatenate([-sinT[0:32], sinT[32:64]], axis=0)  # [64, N]
    cos2 = np.concatenate([cosT, cosT], axis=0)        # [128, N]
    sins2 = np.concatenate([sins, sins], axis=0)       # [128, N]
    # pre-swap 32-row blocks (0<->32, 64<->96): the device multiplies
    # qb[ob] = qf[ib] * sin_sb[ib], so sin_sb[ib] must hold sins2[ob].
    sinsw2 = np.concatenate(
        [sins2[32:64], sins2[0:32], sins2[96:128], sins2[64:96]], axis=0)
    return np.ascontiguousarray(cos2), np.ascontiguousarray(sinsw2)


def kernel(x, w_qkv, w_out, b_out):
    from concourse.bass_utils import run_bass_kernel_spmd

    if "nc" not in _CACHE:
        _CACHE["nc"] = _build()
    nc = _CACHE["nc"]

    bf = ml_dtypes.bfloat16
    cos2, sins2 = _host_tables()
    cos2 = np.ascontiguousarray(cos2.astype(bf))
    sins2 = np.ascontiguousarray(sins2.astype(bf))
    biasb = np.ascontiguousarray(
        np.asarray(b_out, np.float32)[None, :].astype(bf))
    wqkv_b = np.ascontiguousarray(np.asarray(w_qkv, np.float32).astype(bf))
    wout_b = np.ascontiguousarray(np.asarray(w_out, np.float32).astype(bf))

    in_maps = []
    for i in range(N_CORES):
        xi = np.ascontiguousarray(np.asarray(x[i], np.float32).astype(bf))
        in_maps.append({
            "x": xi, "wqkv": wqkv_b, "wout": wout_b,
            "cos2": cos2, "sins2": sins2, "biasb": biasb,
        })

    res = run_bass_kernel_spmd(
        nc, in_maps, list(range(N_CORES)),
        trace=bool(int(os.environ.get("KERNEL_TRACE", "0"))))
    _CACHE["last_result"] = res
    return np.stack([res.results[i]["out"] for i in range(N_CORES)], axis=0)


# revision 74
# speedup vs baseline: 1.0133x; 1.0133x over previous
"""Trainium2 Bass kernel for multi-head attention with RoPE.

Problem: b=8, n=1024, d_model=768, heads=12, dim_head=64.
Strategy: data parallel over batch — each of the 8 NeuronCores handles one
batch element end-to-end (QKV proj + RoPE + attention + out proj). No
collectives needed.

Per-core math (all in transposed [feature, token] layout so every matmul
contraction sits on the partition axis; all matmul operands padded to the
full 128 partitions for full SBUF-stream bandwidth):
  xT   [768,1024]  = x^T             (bf16, via DMA transpose)
  qT   [768,1024]  = Wq^T x^T        then RoPE in bf16 on DVE
  kz   2x[128,1024] per head pair: rotated k rows zero-padded to K=128
  V    [1024,12*128] = x Wv, 128 cols/head: 64 v | ones col | zeros
  per head pair (software-pipelined with the next S/exp step):
    sT[j,i] = sum_d kz[d,j] qT[d,i]  (K=128 contraction, zeros inert)
    pT  = exp(sT / 8)                (no max-subtraction; |S/8| <~ 6)
    oT[128,1024] += Vaug^T-style accum over j tiles; row 64 = softmax
                    denominators (ones column), rows 65+ zeros
    aT = oT[0:64] * bcast(1/oT[64])  (recip_approx + DRAM-broadcast DMA)
  out [1024,768] = aT^T Wout + b
"""

import os
import numpy as np
import ml_dtypes

N = 1024
D = 768
H = 12
DH = 64
E3 = 2304
KT = 6          # number of 128-row tiles of the model dim (768/128)
NT = 8          # number of 128-token tiles (1024/128)
P = 128
N_CORES = 8
VW = 65         # per-head V width incl. ones column

_CACHE = {}


def _build():
    import concourse.bass as bass
    import concourse.mybir as mybir
    import concourse.tile as tile
    from concourse import bacc

    F32 = mybir.dt.float32
    BF16 = mybir.dt.bfloat16
    Exp = mybir.ActivationFunctionType.Exp

    nc = bacc.Bacc("TRN2", target_bir_lowering=False, debug=False,
                   num_devices=N_CORES)

    x = nc.dram_tensor("x", [N, D], BF16, kind="ExternalInput")
    wqkv = nc.dram_tensor("wqkv", [D, E3], BF16, kind="ExternalInput")
    wout = nc.dram_tensor("wout", [D, D], BF16, kind="ExternalInput")
    cos2 = nc.dram_tensor("cos2", [P, N], BF16, kind="ExternalInput")
    sins2 = nc.dram_tensor("sins2", [P, N], BF16, kind="ExternalInput")
    biasb = nc.dram_tensor("biasb", [P, D], F32, kind="ExternalInput")
    out = nc.dram_tensor("out", [N, D], F32, kind="ExternalOutput")

    with tile.TileContext(nc, pool_alloc_mode="queue") as tc:
        import contextlib
        with contextlib.ExitStack() as ctx:
            persist = ctx.enter_context(tc.tile_pool(name="persist", bufs=1))
            qfp = ctx.enter_context(tc.tile_pool(name="qfp", bufs=8))
            scr = ctx.enter_context(tc.tile_pool(name="scr", bufs=5))
            ptp = ctx.enter_context(tc.tile_pool(name="ptp", bufs=4))
            smallp = ctx.enter_context(tc.tile_pool(name="smallp", bufs=2))
            otp = ctx.enter_context(tc.tile_pool(name="otp", bufs=4))
            outp = ctx.enter_context(tc.tile_pool(name="outp", bufs=2))
            dramp = ctx.enter_context(
                tc.tile_pool(name="dram", bufs=2, space="DRAM"))

            # ---- startup loads, ordered so the first V matmul's
            # dependencies (transpose 0 + v-columns) land first
            xT = [persist.tile([P, N], BF16, tag=f"xT{t_i}",
                               name=f"xT_sb{t_i}") for t_i in range(KT)]
            nc.sync.dma_start_transpose(xT[0][:], x[:, 0:P])
            wv_sb = persist.tile([P, KT * D], BF16, tag="wv", name="wv_sb")
            nc.sync.dma_start(
                wv_sb[:].rearrange("p (k e) -> p k e", k=KT),
                wqkv[:, 1536:E3].rearrange("(k p) e -> p k e", p=P))
            for t_i in range(1, KT):
                nc.sync.dma_start_transpose(xT[t_i][:],
                                            x[:, t_i * P:(t_i + 1) * P])
            wqk_sb = persist.tile([P, KT * 1536], BF16, tag="wqk",
                                  name="wqk_sb")
            for k in range(KT):
                nc.sync.dma_start(wqk_sb[:, k * 1536:(k + 1) * 1536],
                                  wqkv[k * P:(k + 1) * P, 0:1536])
            cos_sb = persist.tile([P, N], BF16, tag="cos", name="cos_sb")
            nc.sync.dma_start(cos_sb[:], cos2[:, :])
            sin_sb = persist.tile([P, N], BF16, tag="sin", name="sin_sb")
            nc.sync.dma_start(sin_sb[:], sins2[:, :])
            wo_sb = persist.tile([P, KT * D], BF16, tag="wo", name="wo_sb")
            nc.sync.dma_start(
                wo_sb[:].rearrange("p (k e) -> p k e", k=KT),
                wout[:, :].rearrange("(k p) e -> p k e", p=P))
            bias_sb = persist.tile([P, D], F32, tag="bias", name="bias_sb")
            nc.sync.dma_start(bias_sb[:], biasb[:, :])

            # =======================================================
            # Phase B/C: projections (shared PSUM pool, closed after)
            # =======================================================
            qkT = [persist.tile([P, N], BF16, tag=f"qkT{m}", name=f"qkT_sb{m}")
                   for m in range(6)]
            # zero-padded K tiles: kz[hp][u] holds head 2hp+u's rotated k
            # rows in their natural 64-row half, zeros in the other half,
            # so S^T matmuls contract a full K=128 (full SBUF stream BW).
            kz = [[persist.tile([P, N], BF16, tag=f"kz{hp}_{u}",
                                name=f"kz_sb{hp}_{u}") for u in range(2)]
                  for hp in range(6)]
            # V tiles padded to 128 cols/head: 64 v-dims | ones | zeros,
            # so PV matmuls write a full M=128 (ones col -> sums row 64).
            vt = [persist.tile([P, H * P], BF16, tag=f"vt{n}", name=f"vt_sb{n}")
                  for n in range(NT)]
            aT = [persist.tile([P, N], BF16, tag=f"aT{e}", name=f"aT_sb{e}")
                  for e in range(KT)]
            for hp in range(6):
                nc.gpsimd.memset(kz[hp][0][DH:P, :], 0.0)
                nc.gpsimd.memset(kz[hp][1][0:DH, :], 0.0)

            # Two dedicated PSUM pools (2 slots x 2 banks each = all 8
            # banks): psS cycles projection / S^T / final tiles, psO holds
            # the two PV accumulators. Separate pools make the slot
            # handoff at head-pair boundaries deterministic: fresh S tiles
            # always reuse S slots (freed by exp), PV accumulators reuse
            # the old accumulators (drained by the oT copy during the
            # first S/exp of the new pair).
            with (tc.tile_pool(name="psS", bufs=2, space="PSUM") as psum,
                  tc.tile_pool(name="psO", bufs=2, space="PSUM") as psumO):
                # ---- V projection into per-head 65-wide layout ----
                for ni in range(NT):
                    vpool, vtag = ((psum, "ps") if ni % 2 == 0
                                   else (psumO, "ops"))
                    ps = vpool.tile([P, N], F32, tag=vtag, name="ps_v")
                    for (c0, cw) in ((0, 512), (512, 256)):
                        for k in range(KT):
                            nc.tensor.matmul(
                                ps[:, c0:c0 + cw],
                                lhsT=xT[k][:, ni * P:(ni + 1) * P],
                                rhs=wv_sb[:, k * D + c0:k * D + c0 + cw],
                                start=(k == 0), stop=(k == KT - 1))
                    # scatter copy into head-strided slots (+ones col)
                    dst8 = vt[ni][:, 0:8 * P].rearrange(
                        "p (h j) -> p h j", j=P)[:, :, 0:DH]
                    src8 = ps[:, 0:512].rearrange("p (h j) -> p h j", j=DH)
                    nc.scalar.copy(dst8, src8)
                    dst4 = vt[ni][:, 8 * P:12 * P].rearrange(
                        "p (h j) -> p h j", j=P)[:, :, 0:DH]
                    src4 = ps[:, 512:768].rearrange("p (h j) -> p h j", j=DH)
                    nc.scalar.copy(dst4, src4)
                    vre = vt[ni].rearrange("p (h j) -> p h j", j=P)
                    nc.gpsimd.memset(vre[:, :, DH:DH + 1], 1.0)
                    nc.gpsimd.memset(vre[:, :, DH + 1:P], 0.0)

                # ---- q/k projection + RoPE (order: head-pair hp first
                # needs tiles hp and 6+hp, so emit in that pairing) ----
                for mi, m in enumerate(
                        [t for hp in range(6) for t in (hp, 6 + hp)]):
                    qpool, qtag = ((psum, "ps") if mi % 2 == 0
                                   else (psumO, "ops"))
                    ps = qpool.tile([P, N], F32, tag=qtag, name="ps_qk")
                    for ih in range(2):
                        for k in range(KT):
                            nc.tensor.matmul(
                                ps[:, ih * 512:(ih + 1) * 512],
                                lhsT=wqk_sb[:, k * 1536 + m * P:
                                            k * 1536 + (m + 1) * P],
                                rhs=xT[k][:, ih * 512:(ih + 1) * 512],
                                start=(k == 0), stop=(k == KT - 1))
                    # RoPE in bf16 (DVE 2x_1p mode)
                    qf = qfp.tile([P, N], BF16, tag="qf", name="qf_t")
                    nc.scalar.copy(qf[:], ps[:])
                    qa = scr.tile([P, N], BF16, tag="qa", name="qa_t")
                    nc.vector.tensor_mul(qa[:], qf[:], cos_sb[:])
                    qb = scr.tile([P, N], BF16, tag="qb", name="qb_t")
                    # sin table is pre-swapped on host so in0/in1 share a
                    # base partition (walrus NCC_IBIR297); only the output
                    # is quadrant-shifted. Two of the four quadrant muls
                    # go to the otherwise-idle GpSimd engine — the rope is
                    # DVE-bound otherwise.
                    for blk in range(4):
                        ob = blk * 32
                        ib = (blk ^ 1) * 32  # 0<->32, 64<->96
                        nc.vector.tensor_mul(
                            qb[ob:ob + 32, :], qf[ib:ib + 32, :],
                            sin_sb[ib:ib + 32, :])
                    if m < 6:
                        nc.vector.tensor_add(qkT[m][:], qa[:], qb[:])
                    else:
                        hp = m - 6
                        nc.vector.tensor_add(
                            kz[hp][0][0:DH, :], qa[0:DH, :], qb[0:DH, :])
                        nc.vector.tensor_add(
                            kz[hp][1][DH:P, :], qa[DH:P, :], qb[DH:P, :])

                # ---- attention, two heads (one row-group pair) at a
                # time; software-pipelined one step: emit S/exp of step
                # g+1 before PV of step g so the PE never stalls on the
                # last exp of a head pair (incl. across pair boundaries).
                ss_sb = persist.tile([33, N], F32, tag="ss", name="ss_t")
                nc.gpsimd.memset(ss_sb[0:32, :], 1.0)
                o_ps_all = [[None, None] for _ in range(6)]

                def emit_s_exp(hp, j):
                    qt = qkT[hp]
                    s_ps = [psum.tile([P, N], F32, tag="ps",
                                      name=f"s_ps{u}") for u in range(2)]
                    for ih in range(2):
                        for u in range(2):  # u: head parity
                            nc.tensor.matmul(
                                s_ps[u][:, ih * 512:(ih + 1) * 512],
                                lhsT=kz[hp][u][:, j * P:(j + 1) * P],
                                rhs=qt[:, ih * 512:(ih + 1) * 512],
                                start=True, stop=True)
                    pT = [None, None]
                    for u in range(2):
                        pT[u] = ptp.tile([P, N], BF16, tag="pT",
                                         name=f"pT_t{u}")
                        nc.scalar.activation(pT[u][:], s_ps[u][:], Exp,
                                             scale=0.125)
                    return pT

                def emit_pv(hp, j, pT):
                    o_ps = o_ps_all[hp]
                    if j == 0:
                        o_ps[0] = psumO.tile([P, N], F32, tag="ops",
                                             name="o_ps0")
                        o_ps[1] = psumO.tile([P, N], F32, tag="ops",
                                             name="o_ps1")
                    for u in range(2):
                        h = 2 * hp + u
                        for ih in range(2):
                            nc.tensor.matmul(
                                o_ps[u][:, ih * 512:(ih + 1) * 512],
                                lhsT=vt[j][:, h * P:(h + 1) * P],
                                rhs=pT[u][:, ih * 512:(ih + 1) * 512],
                                start=(j == 0), stop=(j == NT - 1))
                    if j == NT - 1:
                        emit_normalize(hp, o_ps)

                def emit_normalize(hp, o_ps):
                    # copy attn + sums rows out fast (frees o_ps), then
                    # 1/sums + DRAM broadcast + one scale multiply run in
                    # parallel with the next pair's matmuls.
                    oTc = otp.tile([P, N], BF16, tag="oT", name="oT_t")
                    if hp == 5:
                        # last pair: the reciprocal chain gates phase E —
                        # start it first, oTc copies overlap the DMA RT
                        nc.vector.tensor_copy(ss_sb[0:1, :],
                                              o_ps[0][DH:DH + 1, :])
                        nc.vector.tensor_copy(ss_sb[32:33, :],
                                              o_ps[1][DH:DH + 1, :])
                        r_sb = smallp.tile([33, N], F32, tag="r",
                                           name="r_t")
                        nc.vector.reciprocal_approx_fast(r_sb[:], ss_sb[:])
                        nc.scalar.copy(oTc[0:DH, :], o_ps[0][0:DH, :])
                        nc.vector.tensor_copy(oTc[DH:P, :],
                                              o_ps[1][0:DH, :])
                    else:
                        # mid-D: both copies on DVE so ACT's stream stays
                        # pure exp (an ACT copy here waits on the last PV
                        # and delays the next pair's exps by ~1us)
                        nc.vector.tensor_copy(oTc[0:DH, :],
                                              o_ps[0][0:DH, :])
                        nc.vector.tensor_copy(oTc[DH:P, :],
                                              o_ps[1][0:DH, :])
                        nc.vector.tensor_copy(ss_sb[0:1, :],
                                              o_ps[0][DH:DH + 1, :])
                        nc.vector.tensor_copy(ss_sb[32:33, :],
                                              o_ps[1][DH:DH + 1, :])
                        r_sb = smallp.tile([33, N], F32, tag="r",
                                           name="r_t")
                        nc.vector.reciprocal_approx_fast(r_sb[:], ss_sb[:])
                    r_dr = dramp.tile([2, N], F32, tag="rdr", name="rdr_t")
                    nc.sync.dma_start(r_dr[0:1, :], r_sb[0:1, :])
                    nc.sync.dma_start(r_dr[1:2, :], r_sb[32:33, :])
                    rb_sb = smallp.tile([P, N], F32, tag="rb", name="rb_t")
                    nc.sync.dma_start(rb_sb[0:DH, :],
                                      r_dr[0:1, :].broadcast_to([DH, N]))
                    nc.sync.dma_start(rb_sb[DH:P, :],
                                      r_dr[1:2, :].broadcast_to([DH, N]))
                    nc.vector.tensor_mul(aT[hp][:], oTc[:], rb_sb[:])

                steps = [(hp, j) for hp in range(6) for j in range(NT)]
                prev = None
                for st in steps:
                    pT = emit_s_exp(*st)
                    if prev is not None:
                        emit_pv(*prev)
                    prev = (st[0], st[1], pT)
                emit_pv(*prev)

                # ---- output projection + bias (alternate pools so four
                # PSUM regions are available and e<5 accumulation hoists
                # into the last head pair's attention) ----
                for it in range(NT):
                    f_pool = psumO if it % 2 == 0 else psum
                    f_ps = f_pool.tile([P, N], F32,
                                       tag="ops" if it % 2 == 0 else "ps",
                                       name="f_ps")
                    for (c0, cw) in ((0, 512), (512, 256)):
                        for e in range(KT):
                            nc.tensor.matmul(
                                f_ps[:, c0:c0 + cw],
                                lhsT=aT[e][:, it * P:(it + 1) * P],
                                rhs=wo_sb[:, e * D + c0:e * D + c0 + cw],
                                start=(e == 0), stop=(e == KT - 1))
                    o_sb = outp.tile([P, D], F32, tag="osb", name="osb_t")
                    nc.vector.tensor_add(o_sb[:], f_ps[:, 0:D], bias_sb[:])
                    nc.sync.dma_start(out[it * P:(it + 1) * P, :], o_sb[:])

    nc.compile()
    return nc


def _host_tables():
    inv_freq = 1.0 / (10000.0 ** (np.arange(0, DH, 2, dtype=np.float32) / DH))
    t = np.arange(N, dtype=np.float32)
    freqs = np.einsum("i,j->ij", t, inv_freq)          # [N, 32]
    emb = np.concatenate([freqs, freqs], axis=-1)      # [N, 64]
    cosT = np.cos(emb).T.astype(np.float32)            # [64, N]
    sinT = np.sin(emb).T.astype(np.float32)            # [64, N]
    # b-term: out rows 0:32 use -sin (pair d+32), rows 32:64 use +sin
    sins = np.concatenate([-sinT[0:32], sinT[32:64]], axis=0)  # [64, N]
    cos2 = np.concatenate([cosT, cosT], axis=0)        # [128, N]
    sins2 = np.concatenate([sins, sins], axis=0)       # [128, N]
    # pre-swap 32-row blocks (0<->32, 64<->96): the device multiplies
    # qb[ob] = qf[ib] * sin_sb[ib], so sin_sb[ib] must hold sins2[ob].
    sinsw2 = np.concatenate(
        [sins2[32:64], sins2[0:32], sins2[96:128], sins2[64:96]], axis=0)
    return np.ascontiguousarray(cos2), np.ascontiguousarray(sinsw2)


def kernel(x, w_qkv, w_out, b_out):
    from concourse.bass_utils import run_bass_kernel_spmd

    if "nc" not in _CACHE:
        _CACHE["nc"] = _build()
    nc = _CACHE["nc"]

    bf = ml_dtypes.bfloat16
    cos2, sins2 = _host_tables()
    cos2 = np.ascontiguousarray(cos2.astype(bf))
    sins2 = np.ascontiguousarray(sins2.astype(bf))
    biasb = np.ascontiguousarray(
        np.broadcast_to(np.asarray(b_out, np.float32)[None, :], (P, D)))
    wqkv_b = np.ascontiguousarray(np.asarray(w_qkv, np.float32).astype(bf))
    wout_b = np.ascontiguousarray(np.asarray(w_out, np.float32).astype(bf))

    in_maps = []
    for i in range(N_CORES):
        xi = np.ascontiguousarray(np.asarray(x[i], np.float32).astype(bf))
        in_maps.append({
            "x": xi, "wqkv": wqkv_b, "wout": wout_b,
            "cos2": cos2, "sins2": sins2, "biasb": biasb,
        })

    res = run_bass_kernel_spmd(
        nc, in_maps, list(range(N_CORES)),
        trace=bool(int(os.environ.get("KERNEL_TRACE", "0"))))
    _CACHE["last_result"] = res
    return np.stack([res.results[i]["out"] for i in range(N_CORES)], axis=0)


# revision 77
# speedup vs baseline: 1.0394x; 1.0257x over previous
"""Trainium2 Bass kernel for multi-head attention with RoPE.

Problem: b=8, n=1024, d_model=768, heads=12, dim_head=64.
Strategy: data parallel over batch — each of the 8 NeuronCores handles one
batch element end-to-end (QKV proj + RoPE + attention + out proj). No
collectives needed.

Per-core math (all in transposed [feature, token] layout so every matmul
contraction sits on the partition axis; all matmul operands padded to the
full 128 partitions for full SBUF-stream bandwidth):
  xT   [768,1024]  = x^T             (bf16, via DMA transpose)
  qT   [768,1024]  = Wq^T x^T        then RoPE in bf16 on DVE
  kz   2x[128,1024] per head pair: rotated k rows zero-padded to K=128
  V    [1024,12*128] = x Wv, 128 cols/head: 64 v | ones col | zeros
  per head pair (software-pipelined with the next S/exp step):
    sT[j,i] = sum_d kz[d,j] qT[d,i]  (K=128 contraction, zeros inert)
    pT  = exp(sT / 8)                (no max-subtraction; |S/8| <~ 6)
    oT[128,1024] += Vaug^T-style accum over j tiles; row 64 = softmax
                    denominators (ones column), rows 65+ zeros
    aT = oT[0:64] * bcast(1/oT[64])  (recip_approx + DRAM-broadcast DMA)
  out [1024,768] = aT^T Wout + b
"""

import os
import numpy as np
import ml_dtypes

N = 1024
D = 768
H = 12
DH = 64
E3 = 2304
KT = 6          # number of 128-row tiles of the model dim (768/128)
NT = 8          # number of 128-token tiles (1024/128)
P = 128
N_CORES = 8
VW = 65         # per-head V width incl. ones column

_CACHE = {}


def _build():
    import concourse.bass as bass
    import concourse.mybir as mybir
    import concourse.tile as tile
    from concourse import bacc

    F32 = mybir.dt.float32
    BF16 = mybir.dt.bfloat16
    Exp = mybir.ActivationFunctionType.Exp

    nc = bacc.Bacc("TRN2", target_bir_lowering=False, debug=False,
                   num_devices=N_CORES)

    x = nc.dram_tensor("x", [N, D], BF16, kind="ExternalInput")
    wqkv = nc.dram_tensor("wqkv", [D, E3], BF16, kind="ExternalInput")
    wout = nc.dram_tensor("wout", [D, D], BF16, kind="ExternalInput")
    cos2 = nc.dram_tensor("cos2", [P, N], BF16, kind="ExternalInput")
    sins2 = nc.dram_tensor("sins2", [P, N], BF16, kind="ExternalInput")
    biasb = nc.dram_tensor("biasb", [P, D], F32, kind="ExternalInput")
    out = nc.dram_tensor("out", [N, D], F32, kind="ExternalOutput")

    with tile.TileContext(nc, pool_alloc_mode="queue") as tc:
        import contextlib
        with contextlib.ExitStack() as ctx:
            persist = ctx.enter_context(tc.tile_pool(name="persist", bufs=1))
            scr = ctx.enter_context(tc.tile_pool(name="scr", bufs=6))
            ptp = ctx.enter_context(tc.tile_pool(name="ptp", bufs=4))
            smallp = ctx.enter_context(tc.tile_pool(name="smallp", bufs=2))
            otp = ctx.enter_context(tc.tile_pool(name="otp", bufs=4))
            outp = ctx.enter_context(tc.tile_pool(name="outp", bufs=2))
            dramp = ctx.enter_context(
                tc.tile_pool(name="dram", bufs=2, space="DRAM"))

            # ---- startup loads, ordered so the first V matmul's
            # dependencies (transpose 0 + v-columns) land first
            xT = [persist.tile([P, N], BF16, tag=f"xT{t_i}",
                               name=f"xT_sb{t_i}") for t_i in range(KT)]
            nc.sync.dma_start_transpose(xT[0][:], x[:, 0:P])
            wv_sb = persist.tile([P, KT * D], BF16, tag="wv", name="wv_sb")
            nc.sync.dma_start(
                wv_sb[:].rearrange("p (k e) -> p k e", k=KT),
                wqkv[:, 1536:E3].rearrange("(k p) e -> p k e", p=P))
            for t_i in range(1, KT):
                nc.sync.dma_start_transpose(xT[t_i][:],
                                            x[:, t_i * P:(t_i + 1) * P])
            wqk_sb = persist.tile([P, KT * 1536], BF16, tag="wqk",
                                  name="wqk_sb")
            for k in range(KT):
                nc.sync.dma_start(wqk_sb[:, k * 1536:(k + 1) * 1536],
                                  wqkv[k * P:(k + 1) * P, 0:1536])
            cos_sb = persist.tile([P, N], BF16, tag="cos", name="cos_sb")
            nc.sync.dma_start(cos_sb[:], cos2[:, :])
            sin_sb = persist.tile([P, N], BF16, tag="sin", name="sin_sb")
            nc.sync.dma_start(sin_sb[:], sins2[:, :])
            wo_sb = persist.tile([P, KT * D], BF16, tag="wo", name="wo_sb")
            nc.sync.dma_start(
                wo_sb[:].rearrange("p (k e) -> p k e", k=KT),
                wout[:, :].rearrange("(k p) e -> p k e", p=P))
            bias_sb = persist.tile([P, D], F32, tag="bias", name="bias_sb")
            nc.sync.dma_start(bias_sb[:], biasb[:, :])

            # =======================================================
            # Phase B/C: projections (shared PSUM pool, closed after)
            # =======================================================
            qkT = [persist.tile([P, N], BF16, tag=f"qkT{m}", name=f"qkT_sb{m}")
                   for m in range(6)]
            # zero-padded K tiles: kz[hp][u] holds head 2hp+u's rotated k
            # rows in their natural 64-row half, zeros in the other half,
            # so S^T matmuls contract a full K=128 (full SBUF stream BW).
            kz = [[persist.tile([P, N], BF16, tag=f"kz{hp}_{u}",
                                name=f"kz_sb{hp}_{u}") for u in range(2)]
                  for hp in range(6)]
            # V tiles padded to 128 cols/head: 64 v-dims | ones | zeros,
            # so PV matmuls write a full M=128 (ones col -> sums row 64).
            vt = [persist.tile([P, H * P], BF16, tag=f"vt{n}", name=f"vt_sb{n}")
                  for n in range(NT)]
            aT = [persist.tile([P, N], BF16, tag=f"aT{e}", name=f"aT_sb{e}")
                  for e in range(KT)]
            for hp in range(6):
                nc.gpsimd.memset(kz[hp][0][DH:P, :], 0.0)
                nc.gpsimd.memset(kz[hp][1][0:DH, :], 0.0)

            # Two dedicated PSUM pools (2 slots x 2 banks each = all 8
            # banks): psS cycles projection / S^T / final tiles, psO holds
            # the two PV accumulators. Separate pools make the slot
            # handoff at head-pair boundaries deterministic: fresh S tiles
            # always reuse S slots (freed by exp), PV accumulators reuse
            # the old accumulators (drained by the oT copy during the
            # first S/exp of the new pair).
            with (tc.tile_pool(name="psS", bufs=2, space="PSUM") as psum,
                  tc.tile_pool(name="psO", bufs=2, space="PSUM") as psumO):
                # ---- V projection into per-head 65-wide layout ----
                for ni in range(NT):
                    vpool, vtag = ((psum, "ps") if ni % 2 == 0
                                   else (psumO, "ops"))
                    ps = vpool.tile([P, N], F32, tag=vtag, name="ps_v")
                    for (c0, cw) in ((0, 512), (512, 256)):
                        for k in range(KT):
                            nc.tensor.matmul(
                                ps[:, c0:c0 + cw],
                                lhsT=xT[k][:, ni * P:(ni + 1) * P],
                                rhs=wv_sb[:, k * D + c0:k * D + c0 + cw],
                                start=(k == 0), stop=(k == KT - 1))
                    # scatter copy into head-strided slots (+ones col)
                    dst8 = vt[ni][:, 0:8 * P].rearrange(
                        "p (h j) -> p h j", j=P)[:, :, 0:DH]
                    src8 = ps[:, 0:512].rearrange("p (h j) -> p h j", j=DH)
                    nc.scalar.copy(dst8, src8)
                    dst4 = vt[ni][:, 8 * P:12 * P].rearrange(
                        "p (h j) -> p h j", j=P)[:, :, 0:DH]
                    src4 = ps[:, 512:768].rearrange("p (h j) -> p h j", j=DH)
                    nc.scalar.copy(dst4, src4)
                    vre = vt[ni].rearrange("p (h j) -> p h j", j=P)
                    nc.gpsimd.memset(vre[:, :, DH:DH + 1], 1.0)
                    nc.gpsimd.memset(vre[:, :, DH + 1:P], 0.0)

                # ---- q/k projection + RoPE (order: head-pair hp first
                # needs tiles hp and 6+hp, so emit in that pairing) ----
                for mi, m in enumerate(
                        [t for hp in range(6) for t in (hp, 6 + hp)]):
                    qpool, qtag = ((psum, "ps") if mi % 2 == 0
                                   else (psumO, "ops"))
                    ps = qpool.tile([P, N], F32, tag=qtag, name="ps_qk")
                    for ih in range(2):
                        for k in range(KT):
                            nc.tensor.matmul(
                                ps[:, ih * 512:(ih + 1) * 512],
                                lhsT=wqk_sb[:, k * 1536 + m * P:
                                            k * 1536 + (m + 1) * P],
                                rhs=xT[k][:, ih * 512:(ih + 1) * 512],
                                start=(k == 0), stop=(k == KT - 1))
                    # RoPE in bf16 (DVE 2x_1p mode)
                    qf = scr.tile([P, N], BF16, tag="qf", name="qf_t")
                    nc.scalar.copy(qf[:], ps[:])
                    qa = scr.tile([P, N], BF16, tag="qa", name="qa_t")
                    nc.vector.tensor_mul(qa[:], qf[:], cos_sb[:])
                    qb = scr.tile([P, N], BF16, tag="qb", name="qb_t")
                    # sin table is pre-swapped on host so in0/in1 share a
                    # base partition (walrus NCC_IBIR297); only the output
                    # is quadrant-shifted. Two of the four quadrant muls
                    # go to the otherwise-idle GpSimd engine — the rope is
                    # DVE-bound otherwise.
                    for blk in range(4):
                        ob = blk * 32
                        ib = (blk ^ 1) * 32  # 0<->32, 64<->96
                        nc.vector.tensor_mul(
                            qb[ob:ob + 32, :], qf[ib:ib + 32, :],
                            sin_sb[ib:ib + 32, :])
                    if m < 6:
                        nc.vector.tensor_add(qkT[m][:], qa[:], qb[:])
                    else:
                        hp = m - 6
                        nc.vector.tensor_add(
                            kz[hp][0][0:DH, :], qa[0:DH, :], qb[0:DH, :])
                        nc.vector.tensor_add(
                            kz[hp][1][DH:P, :], qa[DH:P, :], qb[DH:P, :])

                # ---- attention, two heads (one row-group pair) at a
                # time; software-pipelined one step: emit S/exp of step
                # g+1 before PV of step g so the PE never stalls on the
                # last exp of a head pair (incl. across pair boundaries).
                ss_sb = persist.tile([33, N], F32, tag="ss", name="ss_t")
                nc.gpsimd.memset(ss_sb[0:32, :], 1.0)
                o_ps_all = [[None, None] for _ in range(6)]

                def emit_s_exp(hp, j):
                    qt = qkT[hp]
                    s_ps = [psum.tile([P, N], F32, tag="ps",
                                      name=f"s_ps{u}") for u in range(2)]
                    for ih in range(2):
                        for u in range(2):  # u: head parity
                            nc.tensor.matmul(
                                s_ps[u][:, ih * 512:(ih + 1) * 512],
                                lhsT=kz[hp][u][:, j * P:(j + 1) * P],
                                rhs=qt[:, ih * 512:(ih + 1) * 512],
                                start=True, stop=True)
                    pT = [None, None]
                    for u in range(2):
                        pT[u] = ptp.tile([P, N], BF16, tag="pT",
                                         name=f"pT_t{u}")
                        nc.scalar.activation(pT[u][:], s_ps[u][:], Exp,
                                             scale=0.125)
                    return pT

                def emit_pv(hp, j, pT):
                    o_ps = o_ps_all[hp]
                    if j == 0:
                        o_ps[0] = psumO.tile([P, N], F32, tag="ops",
                                             name="o_ps0")
                        o_ps[1] = psumO.tile([P, N], F32, tag="ops",
                                             name="o_ps1")
                    for u in range(2):
                        h = 2 * hp + u
                        for ih in range(2):
                            nc.tensor.matmul(
                                o_ps[u][:, ih * 512:(ih + 1) * 512],
                                lhsT=vt[j][:, h * P:(h + 1) * P],
                                rhs=pT[u][:, ih * 512:(ih + 1) * 512],
                                start=(j == 0), stop=(j == NT - 1))
                    if j == NT - 1:
                        emit_normalize(hp, o_ps)

                def emit_normalize(hp, o_ps):
                    # copy attn + sums rows out fast (frees o_ps), then
                    # 1/sums + DRAM broadcast + one scale multiply run in
                    # parallel with the next pair's matmuls.
                    oTc = otp.tile([P, N], BF16, tag="oT", name="oT_t")
                    if hp == 5:
                        # last pair: the reciprocal chain gates phase E —
                        # start it first, oTc copies overlap the DMA RT
                        nc.vector.tensor_copy(ss_sb[0:1, :],
                                              o_ps[0][DH:DH + 1, :])
                        nc.vector.tensor_copy(ss_sb[32:33, :],
                                              o_ps[1][DH:DH + 1, :])
                        r_sb = smallp.tile([33, N], F32, tag="r",
                                           name="r_t")
                        nc.vector.reciprocal_approx_fast(r_sb[:], ss_sb[:])
                        nc.scalar.copy(oTc[0:DH, :], o_ps[0][0:DH, :])
                        nc.vector.tensor_copy(oTc[DH:P, :],
                                              o_ps[1][0:DH, :])
                    elif hp == 0:
                        # hp0 only: DVE still carries the phase-B rope
                        # backlog here, so put the u0 copy on ACT to
                        # release the o slots in parallel
                        nc.scalar.copy(oTc[0:DH, :], o_ps[0][0:DH, :])
                        nc.vector.tensor_copy(oTc[DH:P, :],
                                              o_ps[1][0:DH, :])
                        nc.vector.tensor_copy(ss_sb[0:1, :],
                                              o_ps[0][DH:DH + 1, :])
                        nc.vector.tensor_copy(ss_sb[32:33, :],
                                              o_ps[1][DH:DH + 1, :])
                        r_sb = smallp.tile([33, N], F32, tag="r",
                                           name="r_t")
                        nc.vector.reciprocal_approx_fast(r_sb[:], ss_sb[:])
                    else:
                        # mid-D: both copies on DVE so ACT's stream stays
                        # pure exp (an ACT copy here waits on the last PV
                        # and delays the next pair's exps by ~1us)
                        nc.vector.tensor_copy(oTc[0:DH, :],
                                              o_ps[0][0:DH, :])
                        nc.vector.tensor_copy(oTc[DH:P, :],
                                              o_ps[1][0:DH, :])
                        nc.vector.tensor_copy(ss_sb[0:1, :],
                                              o_ps[0][DH:DH + 1, :])
                        nc.vector.tensor_copy(ss_sb[32:33, :],
                                              o_ps[1][DH:DH + 1, :])
                        r_sb = smallp.tile([33, N], F32, tag="r",
                                           name="r_t")
                        nc.vector.reciprocal_approx_fast(r_sb[:], ss_sb[:])
                    r_dr = dramp.tile([2, N], F32, tag="rdr", name="rdr_t")
                    nc.sync.dma_start(r_dr[0:1, :], r_sb[0:1, :])
                    nc.sync.dma_start(r_dr[1:2, :], r_sb[32:33, :])
                    rb_sb = smallp.tile([P, N], F32, tag="rb", name="rb_t")
                    nc.sync.dma_start(rb_sb[0:DH, :],
                                      r_dr[0:1, :].broadcast_to([DH, N]))
                    nc.sync.dma_start(rb_sb[DH:P, :],
                                      r_dr[1:2, :].broadcast_to([DH, N]))
                    nc.vector.tensor_mul(aT[hp][:], oTc[:], rb_sb[:])

                steps = [(hp, j) for hp in range(6) for j in range(NT)]
                prev = None
                for st in steps:
                    pT = emit_s_exp(*st)
                    if prev is not None:
                        emit_pv(*prev)
                    prev = (st[0], st[1], pT)
                emit_pv(*prev)

                # ---- output projection + bias (alternate pools so four
                # PSUM regions are available and e<5 accumulation hoists
                # into the last head pair's attention) ----
                for it in range(NT):
                    f_pool = psumO if it % 2 == 0 else psum
                    f_ps = f_pool.tile([P, N], F32,
                                       tag="ops" if it % 2 == 0 else "ps",
                                       name="f_ps")
                    for (c0, cw) in ((0, 512), (512, 256)):
                        for e in range(KT):
                            nc.tensor.matmul(
                                f_ps[:, c0:c0 + cw],
                                lhsT=aT[e][:, it * P:(it + 1) * P],
                                rhs=wo_sb[:, e * D + c0:e * D + c0 + cw],
                                start=(e == 0), stop=(e == KT - 1))
                    o_sb = outp.tile([P, D], F32, tag="osb", name="osb_t")
                    nc.vector.tensor_add(o_sb[:], f_ps[:, 0:D], bias_sb[:])
                    nc.sync.dma_start(out[it * P:(it + 1) * P, :], o_sb[:])

    nc.compile()
    return nc


def _host_tables():
    inv_freq = 1.0 / (10000.0 ** (np.arange(0, DH, 2, dtype=np.float32) / DH))
    t = np.arange(N, dtype=np.float32)
    freqs = np.einsum("i,j->ij", t, inv_freq)          # [N, 32]
    emb = np.concatenate([freqs, freqs], axis=-1)      # [N, 64]
    cosT = np.cos(emb).T.astype(np.float32)            # [64, N]
    sinT = np.sin(emb).T.astype(np.float32)            # [64, N]
    # b-term: out rows 0:32 use -sin (pair d+32), rows 32:64 use +sin
    sins = np.concatenate([-sinT[0:32], sinT[32:64]], axis=0)  # [64, N]
    cos2 = np.concatenate([cosT, cosT], axis=0)        # [128, N]
    sins2 = np.concatenate([sins, sins], axis=0)       # [128, N]
    # pre-swap 32-row blocks (0<->32, 64<->96): the device multiplies
    # qb[ob] = qf[ib] * sin_sb[ib], so sin_sb[ib] must hold sins2[ob].
    sinsw2 = np.concatenate(
        [sins2[32:64], sins2[0:32], sins2[96:128], sins2[64:96]], axis=0)
    return np.ascontiguousarray(cos2), np.ascontiguousarray(sinsw2)


def kernel(x, w_qkv, w_out, b_out):
    from concourse.bass_utils import run_bass_kernel_spmd

    if "nc" not in _CACHE:
        _CACHE["nc"] = _build()
    nc = _CACHE["nc"]

    bf = ml_dtypes.bfloat16
    cos2, sins2 = _host_tables()
    cos2 = np.ascontiguousarray(cos2.astype(bf))
    sins2 = np.ascontiguousarray(sins2.astype(bf))
    biasb = np.ascontiguousarray(
        np.broadcast_to(np.asarray(b_out, np.float32)[None, :], (P, D)))
    wqkv_b = np.ascontiguousarray(np.asarray(w_qkv, np.float32).astype(bf))
    wout_b = np.ascontiguousarray(np.asarray(w_out, np.float32).astype(bf))

    in_maps = []
    for i in range(N_CORES):
        xi = np.ascontiguousarray(np.asarray(x[i], np.float32).astype(bf))
        in_maps.append({
            "x": xi, "wqkv": wqkv_b, "wout": wout_b,
            "cos2": cos2, "sins2": sins2, "biasb": biasb,
        })

    res = run_bass_kernel_spmd(
        nc, in_maps, list(range(N_CORES)),
        trace=bool(int(os.environ.get("KERNEL_TRACE", "0"))))
    _CACHE["last_result"] = res
    return np.stack([res.results[i]["out"] for i in range(N_CORES)], axis=0)
